# revision 17
# baseline (speedup 1.0000x reference)
"""Multi-head attention (B=4, T=2048, D=1024, H=16) on 8 Trainium2 cores.

Sharding: core c handles (batch b = c//2, head-group g = c%2) — 8 heads,
512 output features. No inter-core communication.

Host-side: rows of K/V masked out by mask_attn and rows of Q masked out by
mask_out are compacted away (their probabilities / outputs are exactly zero
in the reference), then padded. Activations and weight slices are
pre-transposed so every device matmul contracts over the partition dim, and
converted to bf16 (PSUM accumulation is fp32).

Device per core: project K/Q into transposed [feature, token] layout and V
into natural [token, feature] layout; scores^T = K_h @ Q_h^T per head pair,
packed into disjoint PE row groups (concurrent in the PE array); one
ScalarE instruction applies scale + key-padding bias + exp per 2-head PSUM
tile; PV runs the two heads as column-tiled M=64 matmuls into disjoint
partition halves of one PSUM bank. The softmax denominator: exp of the
first key chunk writes the accumulator tile directly, later chunks are
DVE-added in bf16; the 128-partition reduction plus division (and the V
bias add, which commutes with softmax averaging) happen on the host.

Schedule: the key-chunk sweep is emitted in batches of 2 chunks so the PE
runs 4 score matmuls / 4 PV matmuls back-to-back per config (halves the
pipeline-drain tax at tile-config switches). Projections of head-pair j+1
are emission-interleaved into pair j's attention loop. Input DMAs are
issued on two queues (sync carries the critical K/Q path, gpsimd the bulk)
so compute starts as early as possible; output DMAs issue on gpsimd so
they never block the scalar engine's exp stream.
"""

import os
import sys

sys.path.insert(0, "/opt/trn_rl_repo")

import numpy as np
import ml_dtypes
from contextlib import ExitStack

import concourse.bacc as bacc
import concourse.tile as tile
from concourse import mybir
from concourse.bass_utils import run_bass_kernel_spmd

F32 = mybir.dt.float32
BF16 = mybir.dt.bfloat16

HID = 1024
FO = 512          # projection features per core = 8 heads * 64
HPC = 8           # heads per core
NFI = HID // 128  # contraction chunks
N_CORES = 8


def _tiles(total, w):
    out = []
    o = 0
    while o < total:
        tw = min(w, total - o)
        out.append((o, tw))
        o += tw
    return out


def _blocks(total, first, second=0):
    """First block(s) small (fast DMA landing), then 512-wide blocks."""
    if total <= first:
        return [(0, total)]
    out = [(0, first)]
    o = first
    if second and total - o > second:
        out.append((o, second))
        o += second
    return out + [(o + p, w) for (p, w) in _tiles(total - o, 512)]


def _kvblocks(total):
    return [(0, min(128, total))] + [
        (128 + o, w) for (o, w) in _tiles(total - 128, 256)
    ] if total > 128 else [(0, total)]


def _build(TQ, TK):
    NTK = -(-TK // 128)
    TQT = _blocks(TQ, 256)
    KQB_K = _kvblocks(TK)
    VB = _kvblocks(TK)

    nc = bacc.Bacc("TRN2", target_bir_lowering=False, debug=False)

    qT_d = nc.declare_dram_parameter("qT", [128, NFI * TQ], BF16, isOutput=False)
    kT_d = nc.declare_dram_parameter("kT", [128, NFI * TK], BF16, isOutput=False)
    vT_d = nc.declare_dram_parameter("vT", [128, NFI * TK], BF16, isOutput=False)
    # jf-major weight layouts so per-pair slices are contiguous DMA pieces
    wqT_d = nc.declare_dram_parameter("wqT", [128, 4 * NFI * 128], BF16, isOutput=False)
    wkT_d = nc.declare_dram_parameter("wkT", [128, 4 * NFI * 128], BF16, isOutput=False)
    wvT_d = nc.declare_dram_parameter("wvT", [128, 2 * NFI * 256], BF16, isOutput=False)
    bq_d = nc.declare_dram_parameter("bq", [128, 4], F32, isOutput=False)
    bk_d = nc.declare_dram_parameter("bk", [128, 4], F32, isOutput=False)
    biask_d = nc.declare_dram_parameter("biask", [128, NTK], F32, isOutput=False)
    F16 = mybir.dt.float16
    out_d = nc.declare_dram_parameter("out", [128, 4, TQ], F16, isOutput=True)
    dacc_d = nc.declare_dram_parameter("dacc", [128, 4, 2, TQ], BF16, isOutput=True)

    Exp = mybir.ActivationFunctionType.Exp
    Add = mybir.AluOpType.add

    with tile.TileContext(nc) as tc, ExitStack() as ctx:
        res = ctx.enter_context(tc.tile_pool(name="res", bufs=1))
        qhT = res.tile([128, 4, TQ], BF16)        # [fo%128, pair, t]
        khT = res.tile([128, 4, TK], BF16)
        vh = res.tile([128, NTK, HPC, 64], BF16)  # [t%128, t//128, head, dh]
        biask_sb = res.tile([128, NTK], F32)
        bq_sb = res.tile([128, 4], F32)
        bk_sb = res.tile([128, 4], F32)
        kT_sb = res.tile([128, NFI * TK], BF16)
        qT_sb = res.tile([128, NFI * TQ], BF16)
        vT_sb = res.tile([128, NFI * TK], BF16)
        wq_sb = res.tile([128, 4, NFI, 128], BF16)
        wk_sb = res.tile([128, 4, NFI, 128], BF16)
        wv_sb = res.tile([128, 2, NFI, 256], BF16)

        def tview(sb, t0, tw, blocks):
            # token blocks packed [c, t] along the free dim
            for (b0, bw) in blocks:
                if b0 <= t0 and t0 + tw <= b0 + bw:
                    blk = sb[:, b0 * NFI:(b0 + bw) * NFI].rearrange(
                        "p (c t) -> p c t", c=NFI
                    )
                    return blk[:, :, t0 - b0:t0 - b0 + tw]
            raise AssertionError((t0, tw, blocks))

        def _stream(eng, sb, dd, t0, tw):
            eng.dma_start(
                sb[:, t0 * NFI:(t0 + tw) * NFI],
                dd[:, t0 * NFI:(t0 + tw) * NFI],
            )

        # DMA issue: scalar gets only the 3 tiny constants (so its first
        # ACTIVATE isn't stuck behind serial DMA issues). Everything else
        # streams on ONE queue (sync) in exact consumption order of the
        # it-major schedule: pair-0 weights + all qT up front, then the
        # interleaved kT/vT chunk stream paced to the key-chunk rows, then
        # pairs 1-3 weights (consumed via prefetch late in pair 0).
        W1 = NFI * 128
        nc.scalar.dma_start(bk_sb[:], bk_d[:])
        nc.scalar.dma_start(bq_sb[:], bq_d[:])
        nc.scalar.dma_start(biask_sb[:], biask_d[:])
        nc.sync.dma_start(
            wk_sb[:, 0].rearrange("p c n -> p (c n)"), wkT_d[:, 0:W1]
        )
        _stream(nc.sync, kT_sb, kT_d, *KQB_K[0])
        nc.sync.dma_start(
            wq_sb[:, 0].rearrange("p c n -> p (c n)"), wqT_d[:, 0:W1]
        )
        _stream(nc.sync, qT_sb, qT_d, *TQT[0])
        if len(KQB_K) > 1:
            _stream(nc.sync, kT_sb, kT_d, *KQB_K[1])
        for i in range(1, len(TQT)):
            _stream(nc.sync, qT_sb, qT_d, *TQT[i])
        nc.sync.dma_start(
            wv_sb[:, 0].rearrange("p c n -> p (c n)"), wvT_d[:, 0:NFI * 256]
        )
        _stream(nc.sync, vT_sb, vT_d, *VB[0])
        for i in range(2, len(KQB_K)):
            _stream(nc.sync, kT_sb, kT_d, *KQB_K[i])
            _stream(nc.sync, vT_sb, vT_d, *VB[i - 1])
            if i == 3:
                nc.sync.dma_start(
                    wk_sb[:, 1].rearrange("p c n -> p (c n)"),
                    wkT_d[:, W1:2 * W1],
                )
                nc.sync.dma_start(
                    wq_sb[:, 1].rearrange("p c n -> p (c n)"),
                    wqT_d[:, W1:2 * W1],
                )
        _stream(nc.sync, vT_sb, vT_d, *VB[len(VB) - 1])
        if len(KQB_K) <= 3:
            nc.sync.dma_start(
                wk_sb[:, 1].rearrange("p c n -> p (c n)"), wkT_d[:, W1:2 * W1]
            )
            nc.sync.dma_start(
                wq_sb[:, 1].rearrange("p c n -> p (c n)"), wqT_d[:, W1:2 * W1]
            )
        nc.sync.dma_start(
            wv_sb[:, 1].rearrange("p c n -> p (c n)"), wvT_d[:, NFI * 256:]
        )
        nc.sync.dma_start(
            wk_sb[:, 2:4].rearrange("p j c n -> p (j c n)"), wkT_d[:, 2 * W1:]
        )
        nc.sync.dma_start(
            wq_sb[:, 2:4].rearrange("p j c n -> p (j c n)"), wqT_d[:, 2 * W1:]
        )

        # trigger the exp table-set load (~2.7us) before the first real exp
        warm = res.tile([1, 4], F32)
        nc.scalar.activation(warm[:], bq_sb[0:1, 0:4], Exp, scale=0.0)

        # PSUM budget (8 banks of 2KB): scps 2x[128,2,512]f32 = 4 banks,
        # otps 3x[128,512]f32 = 3 banks (one live PV accumulator per query
        # block, all key chunks of a pair sweep through them), ppj 1 bank.
        ppj = ctx.enter_context(tc.tile_pool(name="ppj", bufs=1, space="PSUM"))
        scps = ctx.enter_context(tc.tile_pool(name="scps", bufs=2, space="PSUM"))
        otps = ctx.enter_context(tc.tile_pool(name="otps", bufs=3, space="PSUM"))
        probs_pool = ctx.enter_context(tc.tile_pool(name="probs", bufs=5))
        dacc_pool = ctx.enter_context(tc.tile_pool(name="dacc", bufs=6))
        park_pool = ctx.enter_context(tc.tile_pool(name="park", bufs=3))

        # Projection generators yield every 2 contraction chunks (4 pulls
        # per block) so the pull sites can spread a block's matmuls across
        # a row — the single ppj buffer's evac then has ~2 pull-sites of
        # slack before the next block's first matmul needs it.
        def gen_kq_proj(jf, src_sb, w_sb, b_sb, dst, blocks):
            """Projection of feature tile jf (one head pair), [fo, t] layout."""
            for (t0, tw) in blocks:
                ps = ppj.tile([128, 512], F32, name="pjps")
                tv = tview(src_sb, t0, tw, blocks)
                for c in range(NFI):
                    nc.tensor.matmul(
                        ps[:, :tw],
                        w_sb[:, jf, c, :],
                        tv[:, c, :],
                        start=(c == 0), stop=(c == NFI - 1),
                    )
                    if c in (1, 3, 5):
                        yield
                nc.vector.tensor_scalar_add(
                    dst[:, jf, t0:t0 + tw], ps[:, :tw], b_sb[:, jf:jf + 1]
                )
                yield

        def gen_v_proj(h0, nh):
            """V projection for heads h0 .. h0+nh-1, natural layout."""
            half, f0 = h0 // 4, (h0 % 4) * 64
            for it in range(NTK):
                iw = min(128, TK - it * 128)
                ps = ppj.tile([128, 512], F32, name="pjps")
                tvv = tview(vT_sb, it * 128, iw, VB)
                for c in range(NFI):
                    nc.tensor.matmul(
                        ps[0:iw, :64 * nh],
                        tvv[:, c, :],
                        wv_sb[:, half, c, f0:f0 + 64 * nh],
                        start=(c == 0), stop=(c == NFI - 1),
                    )
                    if c in (1, 3, 5):
                        yield
                nc.vector.tensor_copy(
                    vh[0:iw, it, h0:h0 + nh, :],
                    ps[0:iw, :64 * nh].rearrange("p (h d) -> p h d", h=nh),
                )
                yield

        g_k = {j: gen_kq_proj(j, kT_sb, wk_sb, bk_sb, khT, KQB_K)
               for j in range(4)}
        g_q = {j: gen_kq_proj(j, qT_sb, wq_sb, bq_sb, qhT, TQT)
               for j in range(4)}
        g_v = {0: gen_v_proj(0, 4), 2: gen_v_proj(4, 2), 3: gen_v_proj(6, 2)}
        # K blocks are pulled one row before the key-chunk row that first
        # consumes them (pair 0's stream is DMA-landing-paced; later pairs
        # have everything resident, the hooks just spread the work).
        k_first_rows = {b0 // 128: 4 for (b0, bw) in KQB_K}

        iws = [min(128, TK - it * 128) for it in range(NTK)]

        otiles = {}
        pending = []
        cur_o = {}
        cur_da = {}

        def emit_pv(slot):
            j, ti, t0, tw, it = slot
            pr = otiles.pop((j, ti, it))
            o = cur_o[(j, ti)]
            iw = iws[it]
            nc.tensor.matmul(
                o[0:64, :tw], vh[0:iw, it, 2 * j, :], pr[0:iw, 0, :tw],
                start=(it == 0), stop=(it == NTK - 1),
            )
            nc.tensor.matmul(
                o[64:128, :tw], vh[0:iw, it, 2 * j + 1, :], pr[0:iw, 1, :tw],
                start=(it == 0), stop=(it == NTK - 1),
            )
            # Denominator accumulation happens here (2 slots after exp) so
            # the DVE writes to the da tile only after the it==0 PV has
            # consumed it as its probs input (exp(it=0) writes da directly).
            da = cur_da[(j, ti)]
            if it > 0:
                nc.vector.tensor_tensor(
                    da[0:iw, :, :tw], da[0:iw, :, :tw], pr[0:iw, :, :tw], Add
                )
            if it == NTK - 1:
                del cur_da[(j, ti)]
                nc.sync.dma_start(dacc_d[:, j, :, t0:t0 + tw], da[:, :, :tw])
                del cur_o[(j, ti)]
                pk = park_pool.tile([128, 512], F16, name="pk")
                nc.vector.tensor_copy(pk[:, :tw], o[:, :tw])
                nc.gpsimd.dma_start(out_d[:, j, t0:t0 + tw], pk[:, :tw])

        def emit_scores(slot):
            """Scores pair matmuls + exp + denominator accumulation."""
            j, ti, t0, tw, it = slot
            iw = iws[it]
            sp = scps.tile([128, 2, 512], F32, name="sc")
            nc.tensor.matmul(
                sp[0:iw, 0, :tw],
                khT[0:64, j, it * 128:it * 128 + iw],
                qhT[0:64, j, t0:t0 + tw],
                start=True, stop=True,
            )
            nc.tensor.matmul(
                sp[0:iw, 1, :tw],
                khT[64:128, j, it * 128:it * 128 + iw],
                qhT[64:128, j, t0:t0 + tw],
                start=True, stop=True,
            )
            return sp

        def emit_exp(slot, sp):
            j, ti, t0, tw, it = slot
            iw = iws[it]
            if it == 0:
                # exp of the full first chunk (iw==128) writes the
                # denominator accumulator directly; later chunks are
                # DVE-added into it during their lagged PV emission.
                pr = dacc_pool.tile([128, 2, 512], BF16, name="da")
                cur_da[(j, ti)] = pr
                o = otps.tile([128, 512], F32, name="ot")
                cur_o[(j, ti)] = o
            else:
                pr = probs_pool.tile([128, 2, 512], BF16, name="pr")
            nc.scalar.activation(
                pr[0:iw, :, :tw], sp[0:iw, :, :tw], Exp,
                bias=biask_sb[0:iw, it:it + 1], scale=0.125,
            )
            otiles[(j, ti, it)] = pr

        # it-major, row-batched emission: per key-chunk row, the PE queue
        # gets [sc(ti0) pair, sc(ti1) pair] back-to-back (same stationary
        # khT slice), then the projection pulls (2-matmul granularity),
        # then the previous row's 3 PV pairs (same stationary vh), then
        # the deferred sc(ti2) pair — it waits on exp(ti0)'s scps buffer,
        # which is done by then, so the 2-buffer scps never stalls the
        # in-order PE.  exp instructions are emitted right after their
        # scores so the ACT queue is always fed.
        for _ in range(4):
            next(g_k[0], None)   # K block 0 (covers key row 0)
            next(g_q[0], None)   # Q block ti=0
        prev_row = []
        for j in range(4):
            for it in range(NTK):
                # hard pulls this row: V chunk it-1 (consumed by the PV
                # flush at this row's middle) + V chunk it8 on the last
                # row, this pair's K block for the NEXT row, and the next
                # pair's first K block on the last row.
                hard = []
                if it == 0:
                    if j > 0:
                        # Q block ti=0 must precede this pair's first scores
                        for _ in range(4):
                            next(g_q[j], None)
                    hard += [g_q[j]] * (4 * (len(TQT) - 1))
                if j in g_v:
                    if it > 0:
                        hard += [g_v[j]] * 4
                    if it == NTK - 1:
                        hard += [g_v[j]] * 4
                if it + 1 in k_first_rows:
                    hard += [g_k[j]] * k_first_rows[it + 1]
                if it == NTK - 1 and j + 1 < 4:
                    hard += [g_k[j + 1]] * 4

                row = []
                for ti in range(len(TQT)):
                    t0, tw = TQT[ti]
                    row.append((j, ti, t0, tw, it))
                sp0 = emit_scores(row[0]); emit_exp(row[0], sp0)
                if it == 0 and len(TQT) > 1:
                    # Q block ti=1 just-in-time before its first scores
                    for _ in range(4):
                        next(hard.pop(0), None)
                if len(row) > 1:
                    sp1 = emit_scores(row[1]); emit_exp(row[1], sp1)
                while hard:
                    next(hard.pop(0), None)
                for s in prev_row:
                    emit_pv(s)
                if len(row) > 2:
                    sp2 = emit_scores(row[2]); emit_exp(row[2], sp2)
                prev_row = row
        for s in prev_row:
            emit_pv(s)

    nc.finalize()
    return nc


def _swz_act(x, blocks):
    """[T, HID] -> [128, NFI*T] packed as token blocks of [NFI, tw]."""
    T = x.shape[0]
    xt = np.ascontiguousarray(x.T).reshape(NFI, 128, T).transpose(1, 0, 2)
    bl = [xt[:, :, t0:t0 + tw].reshape(128, -1) for (t0, tw) in blocks]
    return np.concatenate(bl, axis=1).astype(ml_dtypes.bfloat16)


def _swz_w_jf(w):
    """[FO, HID] -> [128, 4*NFI*128], jf-major: [p, jf, c, m]."""
    wt = w.reshape(4, 128, NFI, 128).transpose(3, 0, 2, 1)
    return np.ascontiguousarray(wt.reshape(128, 4 * NFI * 128)).astype(
        ml_dtypes.bfloat16
    )


def _swz_w_half(w):
    """[FO, HID] -> [128, 2*NFI*256], half-major: [p, half, c, m]."""
    wt = w.reshape(2, 256, NFI, 128).transpose(3, 0, 2, 1)
    return np.ascontiguousarray(wt.reshape(128, 2 * NFI * 256)).astype(
        ml_dtypes.bfloat16
    )


def kernel(q, k, v, Wq, bq, Wk, bk, Wv, bv, mask_attn, mask_out):
    q = np.asarray(q, np.float32)
    k = np.asarray(k, np.float32)
    v = np.asarray(v, np.float32)
    Wq = np.asarray(Wq, np.float32)
    Wk = np.asarray(Wk, np.float32)
    Wv = np.asarray(Wv, np.float32)
    bq = np.asarray(bq, np.float32)
    bk = np.asarray(bk, np.float32)
    bv = np.asarray(bv, np.float32)
    mask_attn = np.asarray(mask_attn)
    mask_out = np.asarray(mask_out)

    B, T, _ = q.shape
    idxk = [np.flatnonzero(mask_attn[b]) for b in range(B)]
    idxq = [np.flatnonzero(mask_out[b]) for b in range(B)]
    TK = max(128, -(-max(len(i) for i in idxk) // 8) * 8)
    TQ = max(256, -(-max(len(i) for i in idxq) // 8) * 8)
    NTK = -(-TK // 128)
    TQT = _blocks(TQ, 256)
    KQB_K = _kvblocks(TK)
    VB = _kvblocks(TK)

    nc = _build(TQ, TK)

    in_maps = []
    for c in range(N_CORES):
        b, g = c // 2, c % 2
        sl = slice(g * FO, (g + 1) * FO)
        nk, nq = len(idxk[b]), len(idxq[b])
        qc = np.zeros((TQ, HID), np.float32)
        qc[:nq] = q[b][idxq[b]]
        kc = np.zeros((TK, HID), np.float32)
        kc[:nk] = k[b][idxk[b]]
        vc = np.zeros((TK, HID), np.float32)
        vc[:nk] = v[b][idxk[b]]
        biask = np.full(TK, -30000.0, np.float32)
        biask[:nk] = 0.0
        in_maps.append({
            "qT": _swz_act(qc, TQT),
            "kT": _swz_act(kc, KQB_K),
            "vT": _swz_act(vc, VB),
            "wqT": _swz_w_jf(Wq[sl]),
            "wkT": _swz_w_jf(Wk[sl]),
            "wvT": _swz_w_half(Wv[sl]),
            "bq": np.ascontiguousarray(bq[sl].reshape(4, 128).T),
            "bk": np.ascontiguousarray(bk[sl].reshape(4, 128).T),
            "biask": np.ascontiguousarray(
                np.pad(biask, (0, NTK * 128 - TK), constant_values=-30000.0)
                .reshape(NTK, 128).T
            ),
        })

    trace_dir = os.environ.get("KERNEL_TRACE_DIR")
    if trace_dir:
        res = run_bass_kernel_spmd(
            nc, in_maps, list(range(N_CORES)), trace=True, tmpdir=trace_dir
        )
        print(f"HW exec time: {res.exec_time_ns} ns")
    else:
        res = run_bass_kernel_spmd(nc, in_maps, list(range(N_CORES)))

    out_full = np.zeros((B, T, HID), np.float32)
    for c in range(N_CORES):
        b, g = c // 2, c % 2
        nq = len(idxq[b])
        u = res.results[c]["out"].astype(np.float32)  # [128, 4, TQ]
        da = res.results[c]["dacc"]        # [128, 4, 2, TQ] bf16
        denom = da.astype(np.float32).sum(axis=0)  # [4, 2, TQ]
        o_all = np.empty((nq, FO), np.float32)
        for j in range(4):
            blk = u[:, j, :nq]             # [128, nq]: h=2j rows 0:64, 2j+1 rows 64:128
            blk = blk / np.repeat(denom[j, :, :nq], 64, axis=0)
            o_all[:, j * 128:(j + 1) * 128] = blk.T
        o_all += bv[g * FO:(g + 1) * FO][None, :]
        out_full[b, idxq[b], g * FO:(g + 1) * FO] = o_all
    return out_full
